# revision 1
# baseline (speedup 1.0000x reference)
"""Trainium2 Bass kernel for a dense transformer block (pre-LN MHA + GELU MLP).

Problem: x[8, 1024, 768]; per-core D=768, H=12 heads, DH=64, FF=3072.
Sharding: data-parallel over batch — the 8 batch elements map 1:1 onto the 8
NeuronCores; no collectives. Each core runs an identical SPMD Tile program on
its own [1024, 768] slice. Measured ~0.50 ms/run on hardware for all 8 cores
(steady-state slope of an 8x-unrolled program); TimelineSim models 374 us
with PE busy ~283 us. End-to-end error vs the fp32 jax reference: 1.5e-3
(absmax-relative), dominated by bf16 operand rounding.

Per-core dataflow (P=128 partitions; activations ping-pong between normal
[tok, d] and transposed [d, tok] layouts so every matmul contracts over the
partition dim and no large transposes are ever needed):
  LN1: stats via bn_stats in normal layout (DVE); rstd via ACT Sqrt + DVE
    reciprocal (ramp phase, ACT idle); xc=(x-mu)*rstd fused to bf16; PE
    transposes 128x128 tiles of xc -> xhatT [d, tok] bf16.
  QKV: qT/kT = W.T @ xhatT (weights stationary, transposed out; bias fused
    into the PSUM->SBUF copy); v = xhatT.T @ Wv (normal out) packed as
    [tok, 12, 65] with a ones column per head.
  Attention per head pair hp (head 2hp on partitions 0-63, 2hp+1 on 64-127):
    scoresT[j,i] = khT.T @ qhT (K=64 slices of kT/qT) for both heads into one
    2-bank PSUM tile; a single ACT Exp call per j covers both heads (no max
    subtraction — |s| < 9 so fp32 exp is safe); ctxU^T plus softmax row-sum
    come from one M=65 matmul with lhsT=[v_h | ones]. Normalization happens
    off-PSUM: copy to f32 SBUF, exact DVE reciprocal of row 64, broadcast by
    bouncing the row through a DRAM tile (step-0 partition APs are DRAM-only)
    and a fused bf16 multiply; head 2hp+1 reaches partitions 64-127 of ctxT
    via a small SBUF->SBUF DMA (engines cannot shift partitions; DMA can).
  Wo: attn = ctxT.T @ Wo (normal out) with the x residual fused into the
    PSUM drain; += bo' on GpSimd. LN2 = LN1 but rstd computed with a
    bit-trick seed + 2 Newton steps on DVE so the ACT Exp table set is never
    swapped mid-attention (Sqrt lives in a different ACT table set; a table
    switch costs ~2.7us and the scheduler interleaves LN2 with attention).
  fc1 = W1.T @ x2hatT with bias+GELU fused on ACT (one Gelu table load);
  fc2 = gT.T @ W2 (normal out) with the x2 residual fused -> out.
  Emission order pipelines Wo/LN2 per 512-token chunk between the two
  attention chunks; PSUM is one pool with static tags (mm=2, s=2x2banks,
  ctx=1+1 of 8 banks).

Host-side folds (exact algebra, no approximation): LN gains into the next
weights (Wq' = diag(g1) Wq / 8, bq' = (b1@Wq + bq)/8, same for k/v without
the 1/8), v-bias through Wo into bo' (softmax rows sum to 1), W1' =
diag(g2) W1, b1' = b2ln@W1 + b1; per-feature bias rows are passed
pre-broadcast as [128, D]. Weights are cast to bf16 on host; all matmul
accumulation is fp32 in PSUM.
"""

from contextlib import ExitStack

import numpy as np
import ml_dtypes

import concourse.bass as bass
import concourse.mybir as mybir
from concourse import bacc
from concourse.tile import TileContext
from concourse.masks import make_identity
from concourse.bass_utils import run_bass_kernel_spmd

f32 = mybir.dt.float32
bf16 = mybir.dt.bfloat16
AF = mybir.ActivationFunctionType
ALU = mybir.AluOpType
ts = bass.ts

B = 8
N = 1024
D = 768
H = 12
DH = 64
FF = 3072
EPS = 1e-6
P = 128
NT = N // P    # 8 token tiles
DT = D // P    # 6 d tiles
FT = FF // P   # 24 ff tiles
CW = 512       # free-dim chunk (one PSUM bank of fp32)
NC_CHUNKS = N // CW  # 2
NCORES = 8

_PROGRAM = None
_TAPS = frozenset()
_tap_handles = {}
_W1TILES = []


def _tap(nc, name, aps):
    if name not in _TAPS:
        return
    shape = [len(aps)] + list(aps[0].shape)
    dt = aps[0].dtype
    h = nc.declare_dram_parameter(f"dbg_{name}", shape, dt, True)
    _tap_handles[f"dbg_{name}"] = shape
    for i, ap in enumerate(aps):
        nc.sync.dma_start(out=h[i], in_=ap)


def _bcast_ap(ap_row, parts):
    """AP reading one (DRAM) row broadcast across `parts` partitions."""
    return bass.AP(tensor=ap_row.tensor, offset=ap_row.offset,
                   ap=[[0, parts]] + list(ap_row.ap[1:]))


def _build_program():
    nc = bacc.Bacc("TRN2", target_bir_lowering=False, debug=False,
                   num_devices=NCORES)

    xd = nc.declare_dram_parameter("x", [N, D], f32, False)
    wqd = nc.declare_dram_parameter("wq", [D, D], bf16, False)
    wkd = nc.declare_dram_parameter("wk", [D, D], bf16, False)
    wvd = nc.declare_dram_parameter("wv", [D, D], bf16, False)
    wod = nc.declare_dram_parameter("wo", [D, D], bf16, False)
    w1d = nc.declare_dram_parameter("w1", [D, FF], bf16, False)
    w2d = nc.declare_dram_parameter("w2", [FF, D], bf16, False)
    bqd = nc.declare_dram_parameter("bq", [P, DT], f32, False)
    bkd = nc.declare_dram_parameter("bk", [P, DT], f32, False)
    b1d = nc.declare_dram_parameter("b1", [P, FT], f32, False)
    bobd = nc.declare_dram_parameter("bob", [P, D], f32, False)
    b2bd = nc.declare_dram_parameter("b2b", [P, D], f32, False)
    outd = nc.declare_dram_parameter("out", [N, D], f32, True)

    with TileContext(nc) as tc:
        _emit_body(nc, tc, xd, wqd, wkd, wvd, wod, w1d, w2d,
                   bqd, bkd, b1d, bobd, b2bd, outd)
    nc.compile()
    return nc


def _ln_tile(nc, pools, x_tile, xhatT, consts, ident, t, tag,
             act_sqrt=False):
    magic_t, eps_t = consts
    """LN stats (DVE) + Newton rsqrt (GpSimd, keeps ACT free of table
    swaps) + center/scale (DVE, bf16) + PE transposes for one token tile;
    writes the t-th column block of each xhatT[j]."""
    ln_pool, xc_pool, ps_main = pools
    stats = ln_pool.tile([P, 3, 6], f32, tag=f"{tag}stats", name=f"{tag}st")
    for s3 in range(3):
        nc.vector.bn_stats(out=stats[:, s3, :],
                           in_=x_tile[:, s3 * 256:(s3 + 1) * 256])
    mv = ln_pool.tile([P, 2], f32, tag=f"{tag}mv", name=f"{tag}mv")
    nc.vector.bn_aggr(out=mv, in_=stats)
    # rstd = rsqrt(var+eps). For LN1 (ramp phase, ACT idle, no Exp yet)
    # use ACT Sqrt + DVE reciprocal: shorter DVE chain. For LN2
    # (interleaved with attention) use a bit-trick seed + 2 Newton steps
    # on DVE so the ACT Exp table set is never swapped out.
    v = ln_pool.tile([P, 1], f32, tag=f"{tag}v", name=f"{tag}v")
    if act_sqrt:
        nc.scalar.activation(out=v, in_=mv[:, 1:2], func=AF.Sqrt,
                             bias=eps_t[:, 0:1])
        rstd = ln_pool.tile([P, 1], f32, tag=f"{tag}rs", name=f"{tag}rs")
        nc.vector.reciprocal(out=rstd, in_=v)
        xc = xc_pool.tile([P, D], bf16, tag=f"{tag}xc", name=f"{tag}xc")
        nc.vector.tensor_scalar(out=xc, in0=x_tile, scalar1=mv[:, 0:1],
                                scalar2=rstd, op0=ALU.subtract, op1=ALU.mult)
        for j in range(DT):
            tp = ps_main.tile([P, P], bf16, tag="mm", bufs=2,
                              name=f"{tag}tr")
            nc.tensor.transpose(tp, xc[:, ts(j, P)], ident)
            if (t * DT + j) % 2 == 0:
                nc.vector.tensor_copy(out=xhatT[j][:, ts(t, P)], in_=tp)
            else:
                nc.scalar.copy(out=xhatT[j][:, ts(t, P)], in_=tp)
        return
    nc.vector.tensor_scalar_add(out=v, in0=mv[:, 1:2], scalar1=EPS)
    y = ln_pool.tile([P, 1], f32, tag=f"{tag}y", name=f"{tag}y")
    hb = ln_pool.tile([P, 1], mybir.dt.uint32, tag=f"{tag}hb",
                      name=f"{tag}hb")
    nc.vector.tensor_scalar(out=hb, in0=v.bitcast(mybir.dt.uint32),
                            scalar1=1, scalar2=None,
                            op0=ALU.logical_shift_right)
    nc.vector.scalar_tensor_tensor(out=y.bitcast(mybir.dt.uint32),
                                   in0=magic_t, scalar=0,
                                   in1=hb, op0=ALU.add, op1=ALU.subtract)
    tN = ln_pool.tile([P, 1], f32, tag=f"{tag}tN", name=f"{tag}tN")
    for _ in range(2):
        nc.vector.tensor_mul(out=tN, in0=y, in1=y)
        nc.vector.tensor_mul(out=tN, in0=tN, in1=v)
        nc.vector.tensor_scalar(out=tN, in0=tN, scalar1=-0.5, scalar2=1.5,
                                op0=ALU.mult, op1=ALU.add)
        nc.vector.tensor_mul(out=y, in0=y, in1=tN)
    xc = xc_pool.tile([P, D], bf16, tag=f"{tag}xc", name=f"{tag}xc")
    nc.vector.tensor_scalar(out=xc, in0=x_tile, scalar1=mv[:, 0:1],
                            scalar2=y, op0=ALU.subtract, op1=ALU.mult)
    for j in range(DT):
        tp = ps_main.tile([P, P], bf16, tag="mm", bufs=2, name=f"{tag}tr")
        nc.tensor.transpose(tp, xc[:, ts(j, P)], ident)
        if (t * DT + j) % 2 == 0:
            nc.vector.tensor_copy(out=xhatT[j][:, ts(t, P)], in_=tp)
        else:
            nc.scalar.copy(out=xhatT[j][:, ts(t, P)], in_=tp)


def _emit_body(nc, tc, xd, wqd, wkd, wvd, wod, w1d, w2d,
               bqd, bkd, b1d, bobd, b2bd, outd):
    class Pools:
        """Manual open/close so SBUF/PSUM lifetimes match phase needs."""

        def __init__(self):
            self._cms = {}

        def open(self, name, **kw):
            cm = tc.tile_pool(name=name, **kw)
            pool = cm.__enter__()
            self._cms[name] = cm
            return pool

        def close(self, *names):
            for n in names:
                self._cms.pop(n).__exit__(None, None, None)

        def close_all(self):
            for n in reversed(list(self._cms)):
                self.close(n)

    pl = Pools()
    try:
        _emit_phases(nc, tc, pl, xd, wqd, wkd, wvd, wod, w1d, w2d,
                     bqd, bkd, b1d, bobd, b2bd, outd)
    finally:
        pl.close_all()


def _emit_phases(nc, tc, pl, xd, wqd, wkd, wvd, wod, w1d, w2d,
                 bqd, bkd, b1d, bobd, b2bd, outd):
    constp = pl.open("const", bufs=1)
    persist = pl.open("persist", bufs=1)
    ident = constp.tile([P, P], bf16, name="ident")
    make_identity(nc, ident)
    magic_t = constp.tile([P, 1], mybir.dt.uint32, name="magic")
    nc.vector.memset(magic_t, 0x5f3759df)
    eps_t = constp.tile([P, 1], f32, name="eps")
    nc.vector.memset(eps_t, EPS)
    bq_sb = constp.tile([P, DT], f32, name="bqs")
    nc.sync.dma_start(out=bq_sb, in_=bqd[:, :])
    bk_sb = constp.tile([P, DT], f32, name="bks")
    nc.sync.dma_start(out=bk_sb, in_=bkd[:, :])
    b1_sb = constp.tile([P, FT], f32, name="b1s")
    nc.sync.dma_start(out=b1_sb, in_=b1d[:, :])
    bo_b = constp.tile([P, D], f32, name="bob")
    nc.sync.dma_start(out=bo_b, in_=bobd[:, :])
    b2_b = constp.tile([P, D], f32, name="b2b")
    nc.sync.dma_start(out=b2_b, in_=b2bd[:, :])

    x2_sb = [persist.tile([P, D], f32, tag=f"x2_{t}", name=f"x2_{t}")
             for t in range(NT)]

    ln_pool = pl.open("ln", bufs=4)
    xc_pool = pl.open("xc", bufs=3)
    # One PSUM pool for the whole body; static bank budget (8):
    #   mm: 2 (transposes, qkv/Wo/fc groups)  s: 2x2 (scores + kv groups)
    #   c0/c1: 1+1 (ctx+rowsum accumulators)
    ps_main = pl.open("ps_main", bufs=1, space="PSUM")
    lnpools = (ln_pool, xc_pool, ps_main)

    # Long-lived pools first (stack order: deepest closes last).
    gp = pl.open("gT", bufs=1)
    x2hatp = pl.open("x2hatT", bufs=1)
    w1p = pl.open("w1p", bufs=3)
    qkvp = pl.open("qkv", bufs=1)

    # ---------- Phase 1: LN1 + transpose (x tiles rotate) ----------
    xhatp = pl.open("xhatT", bufs=1)
    xln = pl.open("xln", bufs=4)
    xhatT = [xhatp.tile([P, N], bf16, tag=f"xh{j}", name=f"xh{j}")
             for j in range(DT)]
    for t in range(NT):
        xt = xln.tile([P, D], f32, tag="xln", name="xln")
        nc.sync.dma_start(out=xt, in_=xd[ts(t, P), :])
        _ln_tile(nc, lnpools, xt, xhatT, (magic_t, eps_t), ident, t,
                 "l1", act_sqrt=True)

    # ---------- Phase 2: QKV projections ----------
    qT = [qkvp.tile([P, N], bf16, tag=f"q{m}", name=f"q{m}")
          for m in range(DT)]
    kT = [qkvp.tile([P, N], bf16, tag=f"k{m}", name=f"k{m}")
          for m in range(DT)]
    v3 = [qkvp.tile([P, H, DH + 1], bf16, tag=f"v{t}", name=f"v{t}")
          for t in range(NT)]
    wp = pl.open("wqkv", bufs=1)
    wq_sb = [wp.tile([P, D], bf16, tag=f"wq{j}", name=f"wq{j}")
             for j in range(DT)]
    wk_sb = [wp.tile([P, D], bf16, tag=f"wk{j}", name=f"wk{j}")
             for j in range(DT)]
    wv_sb = [wp.tile([P, D], bf16, tag=f"wv{j}", name=f"wv{j}")
             for j in range(DT)]
    for j in range(DT):
        nc.sync.dma_start(out=wq_sb[j], in_=wqd[ts(j, P), :])
        nc.sync.dma_start(out=wk_sb[j], in_=wkd[ts(j, P), :])
        nc.sync.dma_start(out=wv_sb[j], in_=wvd[ts(j, P), :])
    for m in range(DT):
        for c in range(NC_CHUNKS):
            ps = ps_main.tile([P, CW], f32, tag="mm", bufs=2, name="qps")
            for j in range(DT):
                nc.tensor.matmul(ps, wq_sb[j][:, ts(m, P)],
                                 xhatT[j][:, ts(c, CW)],
                                 start=(j == 0), stop=(j == DT - 1))
            nc.vector.tensor_scalar_add(out=qT[m][:, ts(c, CW)], in0=ps,
                                        scalar1=bq_sb[:, m:m + 1])
            ps = ps_main.tile([P, 2, CW], f32, tag="s", bufs=2,
                              name="kps")[:, 0, :]
            for j in range(DT):
                nc.tensor.matmul(ps, wk_sb[j][:, ts(m, P)],
                                 xhatT[j][:, ts(c, CW)],
                                 start=(j == 0), stop=(j == DT - 1))
            nc.scalar.activation(out=kT[m][:, ts(c, CW)], in_=ps,
                                 func=AF.Identity, bias=bk_sb[:, m:m + 1])
    for t in range(NT):
        nc.vector.memset(v3[t][:, :, DH:DH + 1], 1.0)
        for lo, w in ((0, 512), (512, 256)):
            ps = ps_main.tile([P, 2, CW], f32, tag="s", bufs=2,
                              name="vps")[:, 0, :]
            for j in range(DT):
                nc.tensor.matmul(ps[:, 0:w], xhatT[j][:, ts(t, P)],
                                 wv_sb[j][:, lo:lo + w],
                                 start=(j == 0), stop=(j == DT - 1))
            h0, nh = lo // DH, w // DH
            nc.vector.tensor_copy(
                out=v3[t][:, h0:h0 + nh, 0:DH],
                in_=ps[:, 0:w].rearrange("p (h d) -> p h d", d=DH))
    _tap(nc, "xh", xhatT)
    _tap(nc, "q", qT)
    _tap(nc, "k", kT)
    _tap(nc, "v", v3)
    pl.close("wqkv", "xln", "xhatT")

    # ---------- Phases 3-5, interleaved by token chunk c ----------
    ctxT = [qkvp.tile([P, N], bf16, tag=f"ctx{m}", name=f"ctx{m}")
            for m in range(DT)]
    x2hatT = [x2hatp.tile([P, N], bf16, tag=f"x2h{j}", name=f"x2h{j}")
              for j in range(DT)]
    gT = [gp.tile([P, N], bf16, tag=f"g{m}", name=f"g{m}")
          for m in range(FT)]
    expp = pl.open("expp", bufs=4)
    smp = pl.open("smallp", bufs=3)
    drp = pl.open("dramp", bufs=4, space="DRAM")
    wop = pl.open("wo", bufs=1)
    xrp = pl.open("xresid", bufs=3)
    wo_sb = [wop.tile([P, D], bf16, tag=f"wo{j}", name=f"wo{j}")
             for j in range(DT)]
    for j in range(DT):
        nc.sync.dma_start(out=wo_sb[j], in_=wod[ts(j, P), :])

    def _fc1_all():
        # m-outer, both chunks per W1 tile: each W1 slice is DMA'd once.
        for m in range(FT):
            w1m = w1p.tile([P, DT, P], bf16, tag="w1m", name="w1m")
            nc.sync.dma_start(
                out=w1m,
                in_=w1d[:, ts(m, P)].rearrange("(jt p) f -> p jt f", p=P))
            for c in range(NC_CHUNKS):
                ps = ps_main.tile([P, CW], f32, tag="mm", bufs=2, name="f1")
                for j in range(DT):
                    nc.tensor.matmul(ps, w1m[:, j, :],
                                     x2hatT[j][:, ts(c, CW)],
                                     start=(j == 0), stop=(j == DT - 1))
                nc.scalar.activation(out=gT[m][:, ts(c, CW)], in_=ps,
                                     func=AF.Gelu, bias=b1_sb[:, m:m + 1])

    def _attention_pair(hp, c):
        h0, h1 = 2 * hp, 2 * hp + 1
        cps0 = ps_main.tile([P, CW], f32, tag="c0", bufs=1, name="c0")
        cps1 = ps_main.tile([P, CW], f32, tag="c1", bufs=1, name="c1")
        for j in range(NT):
            sps = ps_main.tile([P, 2, CW], f32, tag="s", bufs=2, name="sps")
            nc.tensor.matmul(sps[:, 0, :], kT[hp][0:DH, ts(j, P)],
                             qT[hp][0:DH, ts(c, CW)], start=True, stop=True)
            nc.tensor.matmul(sps[:, 1, :], kT[hp][DH:P, ts(j, P)],
                             qT[hp][DH:P, ts(c, CW)], start=True, stop=True)
            ee = expp.tile([P, 2, CW], bf16, tag="e", name="ee")
            nc.scalar.activation(out=ee, in_=sps, func=AF.Exp)
            first, last = j == 0, j == NT - 1
            # M=65: col 64 of v3 is ones -> row 64 = softmax row-sum
            nc.tensor.matmul(cps0[0:DH + 1, :], v3[j][:, h0, 0:DH + 1],
                             ee[:, 0, :], start=first, stop=last)
            nc.tensor.matmul(cps1[0:DH + 1, :], v3[j][:, h1, 0:DH + 1],
                             ee[:, 1, :], start=first, stop=last)
        for hh, cps in ((h0, cps0), (h1, cps1)):
            # Drain PSUM to f32 SBUF immediately (frees the bank), then
            # normalize off-PSUM.
            cu = smp.tile([DH + 1, CW], f32, tag=f"cu{hh % 2}", name="cu")
            nc.vector.tensor_copy(out=cu, in_=cps[0:DH + 1, :])
            rb = smp.tile([DH + 1, CW], f32, tag=f"rb{hh % 2}", name="rb")
            nc.vector.reciprocal(out=rb[DH:DH + 1, :],
                                 in_=cu[DH:DH + 1, :])
            drow = drp.tile([1, CW], f32, tag=f"drow{hh % 2}", name="drow")
            nc.sync.dma_start(out=drow, in_=rb[DH:DH + 1, :])
            nc.sync.dma_start(out=rb[0:DH, :],
                              in_=_bcast_ap(drow[0:1, :], DH))
            if hh % 2 == 0:
                nc.vector.tensor_mul(ctxT[hp][0:DH, ts(c, CW)],
                                     cu[0:DH, :], rb[0:DH, :])
            else:
                tmp = smp.tile([DH, CW], bf16, tag="tmp", name="tmp")
                nc.vector.tensor_mul(tmp, cu[0:DH, :], rb[0:DH, :])
                nc.sync.dma_start(out=ctxT[hp][DH:P, ts(c, CW)], in_=tmp)

    def _wo_ln2_tile(t):
        xr = xrp.tile([P, D], f32, tag="xr", name="xr")
        nc.sync.dma_start(out=xr, in_=xd[ts(t, P), :])
        for lo, w in ((0, 512), (512, 256)):
            ps = ps_main.tile([P, 512], f32, tag="mm", bufs=2, name="ops")
            for j in range(DT):
                nc.tensor.matmul(ps[:, 0:w], ctxT[j][:, ts(t, P)],
                                 wo_sb[j][:, lo:lo + w],
                                 start=(j == 0), stop=(j == DT - 1))
            nc.vector.scalar_tensor_tensor(
                out=x2_sb[t][:, lo:lo + w], in0=ps[:, 0:w], scalar=1.0,
                in1=xr[:, lo:lo + w], op0=ALU.mult, op1=ALU.add)
        nc.gpsimd.tensor_add(out=x2_sb[t], in0=x2_sb[t], in1=bo_b)
        _ln_tile(nc, lnpools, x2_sb[t], x2hatT, (magic_t, eps_t),
                 ident, t, "l2")
        nc.gpsimd.tensor_add(out=x2_sb[t], in0=x2_sb[t], in1=b2_b)

    for c in range(NC_CHUNKS):
        for hp in range(H // 2):
            _attention_pair(hp, c)
        for t in range(4 * c, 4 * (c + 1)):
            _wo_ln2_tile(t)

    _tap(nc, "ctx", ctxT)
    _tap(nc, "x2", x2_sb)
    _tap(nc, "x2h", x2hatT)
    _tap(nc, "g", gT)
    pl.close("xresid", "wo", "dramp", "smallp", "expp", "qkv")

    # ---------- Phase 6: w2 prefetch, last fc1 chunk, fc2 ----------
    w2p = pl.open("w2p", bufs=1)
    outp = pl.open("outp", bufs=3)
    w2_sb = [w2p.tile([P, D], bf16, tag=f"w2_{m}", name=f"w2_{m}")
             for m in range(FT)]
    _fc1_all()
    for m in range(FT):
        nc.sync.dma_start(out=w2_sb[m], in_=w2d[ts(m, P), :])
    for t in range(NT):
        ot = outp.tile([P, D], f32, tag="out", name="ot")
        for lo, w in ((0, 512), (512, 256)):
            ps = ps_main.tile([P, 512], f32, tag="mm", bufs=2, name="f2")
            for m in range(FT):
                nc.tensor.matmul(ps[:, 0:w], gT[m][:, ts(t, P)],
                                 w2_sb[m][:, lo:lo + w],
                                 start=(m == 0), stop=(m == FT - 1))
            nc.vector.scalar_tensor_tensor(
                out=ot[:, lo:lo + w], in0=ps[:, 0:w], scalar=1.0,
                in1=x2_sb[t][:, lo:lo + w], op0=ALU.mult, op1=ALU.add)
        nc.sync.dma_start(out=outd[ts(t, P), :], in_=ot)


def _get_program():
    global _PROGRAM
    if _PROGRAM is None:
        _PROGRAM = _build_program()
    return _PROGRAM


def _prepare_host_inputs(inputs):
    f64 = np.float64
    x = np.asarray(inputs["x"], np.float32)
    g1 = np.asarray(inputs["ln1_g"], f64)
    b1l = np.asarray(inputs["ln1_b"], f64)
    g2 = np.asarray(inputs["ln2_g"], f64)
    b2l = np.asarray(inputs["ln2_b"], f64)
    Wq = np.asarray(inputs["Wq"], f64)
    Wk = np.asarray(inputs["Wk"], f64)
    Wv = np.asarray(inputs["Wv"], f64)
    Wo = np.asarray(inputs["Wo"], f64)
    W1 = np.asarray(inputs["W1"], f64)
    W2 = np.asarray(inputs["W2"], f64)
    bq = np.asarray(inputs["bq"], f64)
    bk = np.asarray(inputs["bk"], f64)
    bv = np.asarray(inputs["bv"], f64)
    bo = np.asarray(inputs["bo"], f64)
    b1 = np.asarray(inputs["b1"], f64)
    b2 = np.asarray(inputs["b2"], f64)

    def bf(a):
        return np.ascontiguousarray(a.astype(np.float32)).astype(
            ml_dtypes.bfloat16)

    def col_tile(vec, nt):  # [nt*P] -> [P, nt]
        return np.ascontiguousarray(vec.astype(np.float32).reshape(nt, P).T)

    wq_h = bf(g1[:, None] * Wq * 0.125)
    bq_h = col_tile((b1l @ Wq + bq) * 0.125, DT)
    wk_h = bf(g1[:, None] * Wk)
    bk_h = col_tile(b1l @ Wk + bk, DT)
    wv_h = bf(g1[:, None] * Wv)
    bv_f = b1l @ Wv + bv
    wo_h = bf(Wo)
    bo_f = bo + bv_f @ Wo
    bob_h = np.ascontiguousarray(
        np.broadcast_to(bo_f.astype(np.float32), (P, D)))
    w1_h = bf(g2[:, None] * W1)
    b1_h = col_tile(b2l @ W1 + b1, FT)
    w2_h = bf(W2)
    b2b_h = np.ascontiguousarray(
        np.broadcast_to(b2.astype(np.float32), (P, D)))

    shared = {"wq": wq_h, "wk": wk_h, "wv": wv_h, "wo": wo_h,
              "w1": w1_h, "w2": w2_h, "bq": bq_h, "bk": bk_h,
              "b1": b1_h, "bob": bob_h, "b2b": b2b_h}
    return x, shared


def kernel(**inputs):
    x, shared = _prepare_host_inputs(inputs)
    nc = _get_program()
    in_maps = [dict(shared, x=np.ascontiguousarray(x[c]))
               for c in range(NCORES)]
    import time
    last_err = None
    for attempt in range(3):
        try:
            t0 = time.perf_counter()
            res = run_bass_kernel_spmd(nc, in_maps, list(range(NCORES)))
            t1 = time.perf_counter()
            break
        except Exception as e:  # transient NRT device wedge: retry
            last_err = e
            time.sleep(2.0 * (attempt + 1))
    else:
        raise last_err
    kernel._last_wall_s = t1 - t0
    out = np.stack([res.results[c]["out"] for c in range(NCORES)], axis=0)
    return out.astype(np.float32)



# revision 32
# speedup vs baseline: 1.1789x; 1.1789x over previous
"""Trainium2 Bass kernel for a dense transformer block (pre-LN MHA + GELU MLP).

Problem: x[8, 1024, 768]; per-core D=768, H=12 heads, DH=64, FF=3072.
Sharding: data-parallel over batch - the 8 batch elements map 1:1 onto the 8
NeuronCores; no collectives. Each core runs an identical SPMD Tile program on
its own [1024, 768] slice.

Every GEMM runs as fp8 DoubleRow matmuls (two 128-deep k-subtiles per
instruction at 0.5 cycles/row = 4x bf16 throughput). Precision is allocated
per-GEMM from an ablation study (numpy absmax-rel error vs the fp32
reference; budget 2e-2, this config measures ~8e-3):
  q/k proj:  W-compensated 2-term (W_hi e4m3 + W_lo e5m2), activation e4m3.
             The k bias is dropped exactly (softmax shift invariance); the q
             bias is kept (it varies per key after the transpose).
  v proj/Wo: raw e4m3 (insensitive: attention averaging damps v errors).
  scores:    q,k e4m3, DH=64 contraction packed as 2 subtiles of 32
             partitions; 4 heads share a 128-partition group at bases 32h'.
  softmax:   exact exp on ACT; probs e5m2 (max score ~6.3 so e^s reaches
             ~540: overflows e4m3's 448 max, fits e5m2; no max-subtraction).
  ctx:       probs(e5m2) x v(e4m3) DoubleRow in [query, d] orientation so
             the softmax denominator is per-PARTITION: one reciprocal + one
             stride-0-broadcast multiply per (head, chunk).
  fc1:       3-term compensated (a_hi@W_hi + a_lo@W_hi + a_hi@W_lo); W_lo
             in e5m2 because W_lo values (~3.6% of already-small weights)
             fall below e4m3's subnormal step.
  fc2:       W-compensated 2-term (gelu output raw e4m3 straight from ACT;
             a gelu-side lo term would need two extra full passes over
             24.6k elems/partition on DVE/Pool, which are saturated).
All accumulation is fp32 in PSUM; the residual stream and layernorms are
fp32. Exp and Gelu live in different ACT table sets (~1.3us per switch), so
all of chunk-0's fc1 GELUs are emitted as one block after the last exp.
Measured on hardware: rel err 1.29e-2 (budget 2e-2), matching the numpy
quantization model (1.31e-2); TimelineSim 317us vs the 374us bf16 baseline.
GpSimd cannot touch PSUM (BIR verifier) so every PSUM drain is on DVE/ACT;
fp8 PE transposes need an e4m3 identity and write PSUM 2-byte strided.
"""

from contextlib import ExitStack

import numpy as np
import ml_dtypes

import concourse.bass as bass
import concourse.mybir as mybir
from concourse import bacc
from concourse.tile import TileContext
from concourse.masks import make_identity
from concourse.bass_utils import run_bass_kernel_spmd

f32 = mybir.dt.float32
bf16 = mybir.dt.bfloat16
e4 = mybir.dt.float8e4
e5 = mybir.dt.float8e5
AF = mybir.ActivationFunctionType
ALU = mybir.AluOpType
DR = mybir.MatmulPerfMode.DoubleRow
ts = bass.ts

B = 8
N = 1024
D = 768
H = 12
DH = 64
FF = 3072
EPS = 1e-6
P = 128
NT = N // P    # 8 token tiles
DT = D // P    # 6 d tiles
UT = FF // P   # 24 ff tiles
CW = 512       # query/psum chunk (one PSUM bank of fp32)
NC = N // CW   # 2 chunks
NCORES = 8

_PROGRAM = None
_TAPS = frozenset()
_tap_handles = {}


def _tap(nc, name, aps):
    if name not in _TAPS:
        return
    shape = [len(aps)] + list(aps[0].shape)
    dt = aps[0].dtype
    h = nc.declare_dram_parameter(f"dbg_{name}", shape, dt, True)
    _tap_handles[f"dbg_{name}"] = shape
    for i, ap in enumerate(aps):
        nc.sync.dma_start(out=h[i], in_=ap)


def _bcast_free(ap, n):
    """AP with a stride-0 trailing dim of size n (free-dim broadcast)."""
    return bass.AP(tensor=ap.tensor, offset=ap.offset,
                   ap=list(ap.ap) + [[0, n]])


def _tr_view(pst):
    """[P, 6, 128] fp8 view of a PSUM bank with element step 2 (the PE
    writes fp8 transpose outputs 2-byte strided)."""
    b = pst.bitcast(e4)
    return bass.AP(tensor=b.tensor, offset=b.offset,
                   ap=[list(b.ap[0]), [256, DT], [2, P]])


def _build_program():
    nc = bacc.Bacc("TRN2", target_bir_lowering=False, debug=False,
                   num_devices=NCORES)

    xd = nc.declare_dram_parameter("x", [N, D], f32, False)
    wqkh = nc.declare_dram_parameter("wqkh", [P, DT, 16, 96], e4, False)
    wqkl = nc.declare_dram_parameter("wqkl", [P, DT, 16, 96], e5, False)
    wvd = nc.declare_dram_parameter("wv", [P, DT, D], e4, False)
    wod = nc.declare_dram_parameter("wo", [P, DT, D], e4, False)
    w1hd = nc.declare_dram_parameter("w1h", [UT, P, DT, P], e4, False)
    w1ld = nc.declare_dram_parameter("w1l", [UT, P, DT, P], e5, False)
    w2hd = nc.declare_dram_parameter("w2h", [P, UT, D], e4, False)
    w2ld = nc.declare_dram_parameter("w2l", [P, UT, D], e5, False)
    bqd = nc.declare_dram_parameter("bq", [P, 8], f32, False)
    b1d = nc.declare_dram_parameter("b1", [P, UT], f32, False)
    bobd = nc.declare_dram_parameter("bob", [P, D], f32, False)
    b2bd = nc.declare_dram_parameter("b2b", [P, D], f32, False)
    outd = nc.declare_dram_parameter("out", [N, D], f32, True)

    with TileContext(nc) as tc:
        _emit_body(nc, tc, xd, wqkh, wqkl, wvd, wod, w1hd, w1ld, w2hd, w2ld,
                   bqd, b1d, bobd, b2bd, outd)
    nc.compile()
    return nc


class Pools:
    """Manual open/close so SBUF/PSUM lifetimes match phase needs."""

    def __init__(self, tc):
        self._tc = tc
        self._cms = {}

    def open(self, name, **kw):
        cm = self._tc.tile_pool(name=name, **kw)
        pool = cm.__enter__()
        self._cms[name] = cm
        return pool

    def close(self, *names):
        for n in names:
            self._cms.pop(n).__exit__(None, None, None)

    def close_all(self):
        for n in reversed(list(self._cms)):
            self.close(n)


def _emit_body(nc, tc, *args):
    pl = Pools(tc)
    try:
        _emit_phases(nc, tc, pl, *args)
    finally:
        pl.close_all()


def _emit_phases(nc, tc, pl, xd, wqkh, wqkl, wvd, wod, w1hd, w1ld, w2hd,
                 w2ld, bqd, b1d, bobd, b2bd, outd):
    constp = pl.open("const", bufs=1)
    ident = constp.tile([P, P], e4, name="ident")
    make_identity(nc, ident)
    magic_t = constp.tile([P, 1], mybir.dt.uint32, name="magic")
    nc.vector.memset(magic_t, 0x5f3759df)
    eps_t = constp.tile([P, 1], f32, name="eps")
    nc.vector.memset(eps_t, EPS)
    bq_sb = constp.tile([P, 8], f32, name="bqs")
    b1_sb = constp.tile([P, UT], f32, name="b1s")
    bo_b = constp.tile([P, D], f32, name="bob")
    b2_b = constp.tile([P, D], f32, name="b2b")

    x2p = pl.open("x2p", bufs=1)
    x2_sb = [x2p.tile([P, D], f32, tag=f"x2_{t}", name=f"x2_{t}")
             for t in range(NT)]

    # One PSUM pool, static banks: s 2x2 + c 2x1 + mm 2x1 = 8.
    ps = pl.open("ps", bufs=1, space="PSUM")

    def ps_s():
        return ps.tile([P, 2, CW], f32, tag="s", bufs=2, name="sps")

    def ps_c():
        # One full fp32 bank; viewed as fp8 for transpose groups or as
        # [P, 4, 65] fp32 for ctx accumulation.
        return ps.tile([P, CW], f32, tag="c", bufs=2, name="cps")

    def ps_mm():
        return ps.tile([P, CW], f32, tag="mm", bufs=2, name="mm")

    ln_pool = pl.open("ln", bufs=4)
    xc_pool = pl.open("xc", bufs=2)
    xlnp = pl.open("xln", bufs=2)

    # ---- big SBUF region A: attention-side tensors ----
    bigA = pl.open("bigA", bufs=1)
    qT = [bigA.tile([P, 2, N], e4, tag=f"q{g}", name=f"q{g}")
          for g in range(4)]
    kT = [bigA.tile([P, 2, N], e4, tag=f"k{g}", name=f"k{g}")
          for g in range(4)]
    v3p = [bigA.tile([P, 2, H, DH + 1], e4, tag=f"v{jj}", name=f"v{jj}")
           for jj in range(4)]
    ctx_sb = bigA.tile([P, NT, D], e4, tag="ctxs", name="ctxs")
    ctxT = bigA.tile([P, DT, N], e4, tag="ctxT", name="ctxT")
    wo_sb = bigA.tile([P, DT, D], e4, tag="wo", name="wo")
    eep = pl.open("ee", bufs=5)
    rcp = pl.open("rc", bufs=3)
    xrp = pl.open("xr", bufs=4)
    wqkp = pl.open("wqk", bufs=1)
    xhT = wqkp.tile([P, DT, N], e4, tag="xhT", name="xhT")
    wv_sb = wqkp.tile([P, DT, D], e4, tag="wv", name="wv")
    wqk_h = wqkp.tile([P, DT, 16, 96], e4, tag="wqkh", name="wqkh")
    wqk_l = wqkp.tile([P, DT, 16, 96], e5, tag="wqkl", name="wqkl")
    for jj in range(4):
        nc.vector.memset(v3p[jj][:, :, :, DH:DH + 1], 1.0)

    def _newton_rstd(mv, sfx):
        var = ln_pool.tile([P, 1], f32, tag="v" + sfx, name="v" + sfx)
        nc.vector.tensor_scalar_add(out=var, in0=mv[:, 1:2], scalar1=EPS)
        y = ln_pool.tile([P, 1], f32, tag="y" + sfx, name="y" + sfx)
        hb = ln_pool.tile([P, 1], mybir.dt.uint32, tag="hb" + sfx,
                          name="hb" + sfx)
        nc.vector.tensor_scalar(out=hb, in0=var.bitcast(mybir.dt.uint32),
                                scalar1=1, scalar2=None,
                                op0=ALU.logical_shift_right)
        nc.vector.scalar_tensor_tensor(out=y.bitcast(mybir.dt.uint32),
                                       in0=magic_t, scalar=0,
                                       in1=hb, op0=ALU.add, op1=ALU.subtract)
        tN = ln_pool.tile([P, 1], f32, tag="tN" + sfx, name="tN" + sfx)
        for _ in range(2):
            nc.vector.tensor_mul(out=tN, in0=y, in1=y)
            nc.vector.tensor_mul(out=tN, in0=tN, in1=var)
            nc.vector.tensor_scalar(out=tN, in0=tN, scalar1=-0.5,
                                    scalar2=1.5, op0=ALU.mult, op1=ALU.add)
            nc.vector.tensor_mul(out=y, in0=y, in1=tN)
        return y

    # ---------- LN1 per token tile: stats, fp8 xhat, transposes ----------
    def ln1_tile(t):
        xt = xlnp.tile([P, D], f32, tag="xln", name="xln")
        nc.sync.dma_start(out=xt, in_=xd[ts(t, P), :])
        stats = ln_pool.tile([P, 3, 6], f32, tag="st", name="st")
        for s3 in range(3):
            nc.vector.bn_stats(out=stats[:, s3, :],
                               in_=xt[:, s3 * 256:(s3 + 1) * 256])
        mv = ln_pool.tile([P, 2], f32, tag="mv", name="mv")
        nc.vector.bn_aggr(out=mv, in_=stats)
        rstd = _newton_rstd(mv, "1")
        xc = xc_pool.tile([P, D], e4, tag="xc", name="xc")
        nc.vector.tensor_scalar(out=xc, in0=xt, scalar1=mv[:, 0:1],
                                scalar2=rstd, op0=ALU.subtract, op1=ALU.mult)
        tr = _tr_view(ps_mm())
        for j in range(DT):
            nc.tensor.matmul(tr[:, j:j + 1, :], xc[:, ts(j, P)], ident,
                             is_transpose=True, start=(j == 0),
                             stop=(j == DT - 1), skip_group_check=True)
        nc.vector.tensor_copy(out=xhT[:, :, ts(t, P)], in_=tr)

    for t in range(NT):
        ln1_tile(t)
    # Weight/bias DMAs after the x tiles on the serial DMA pipe, in the
    # order each is first needed.
    nc.sync.dma_start(out=wv_sb, in_=wvd[:, :, :])
    nc.sync.dma_start(out=wqk_h, in_=wqkh[:, :, :, :])
    nc.sync.dma_start(out=wqk_l, in_=wqkl[:, :, :, :])
    nc.sync.dma_start(out=bq_sb, in_=bqd[:, :])
    nc.sync.dma_start(out=wo_sb, in_=wod[:, :, :])
    nc.sync.dma_start(out=bo_b, in_=bobd[:, :])
    nc.sync.dma_start(out=b2_b, in_=b2bd[:, :])
    nc.sync.dma_start(out=b1_sb, in_=b1d[:, :])

    # ---------- QKV ----------
    def v_tile(t):
        for lo, w in ((0, CW), (CW, 256)):
            mm = ps_mm()
            for m2 in range(3):
                nc.tensor.matmul(mm[:, 0:w],
                                 xhT[:, 2 * m2:2 * m2 + 2, ts(t, P)],
                                 wv_sb[:, 2 * m2:2 * m2 + 2, lo:lo + w],
                                 start=(m2 == 0), stop=(m2 == 2),
                                 perf_mode=DR)
            h0, nh = lo // DH, w // DH
            nc.vector.tensor_copy(
                out=v3p[t // 2][:, t % 2, h0:h0 + nh, 0:DH],
                in_=mm[:, 0:w].rearrange("p (h d) -> p h d", d=DH))

    def qk_tile(kind, g, s, c):
        # kind 0 = q (bias added), 1 = k (bias dropped exactly).
        # 3 heads per group at base partitions 0/32/64 (96 used of 128).
        idx = kind * 8 + g * 2 + s
        mm = ps_mm()
        for wsb, first, last in ((wqk_h, True, False), (wqk_l, False, True)):
            for m2 in range(3):
                nc.tensor.matmul(
                    mm[0:96, :], wsb[:, 2 * m2:2 * m2 + 2, idx, :],
                    xhT[:, 2 * m2:2 * m2 + 2, ts(c, CW)],
                    start=(first and m2 == 0), stop=(last and m2 == 2),
                    perf_mode=DR)
        dst = (qT if kind == 0 else kT)[g][0:96, s, ts(c, CW)]
        if kind == 0:
            sl = g * 2 + s
            nc.vector.tensor_scalar_add(out=dst, in0=mm[0:96, :],
                                        scalar1=bq_sb[0:96, sl:sl + 1])
        else:
            nc.vector.tensor_copy(out=dst, in_=mm[0:96, :])

    def qk_group(g):
        for c in range(NC):
            for s in range(2):
                qk_tile(0, g, s, c)
                qk_tile(1, g, s, c)

    # q/k group 0 first: the chunk-0 exp stream starts before v finishes
    # (ctx matmuls wait on v, exps don't).
    for g in range(4):
        qk_group(g)
    for t in range(NT):
        v_tile(t)
    pl.close("wqk")
    # ---- big SBUF region B: MLP-side tensors ----
    bigB = pl.open("bigB", bufs=1)
    x2hT_h = bigB.tile([P, DT, N], e4, tag="x2h", name="x2h")
    x2hT_l = bigB.tile([P, DT, N], e4, tag="x2l", name="x2l")
    w2_h = bigB.tile([P, UT, D], e4, tag="w2h", name="w2h")
    w2_l = bigB.tile([P, UT, D], e5, tag="w2l", name="w2l")
    nc.sync.dma_start(out=w2_h, in_=w2hd[:, :, :])
    nc.sync.dma_start(out=w2_l, in_=w2ld[:, :, :])
    gp = pl.open("gp", bufs=1)
    w1p = pl.open("w1p", bufs=3)
    outp = pl.open("outp", bufs=2)


    _tap(nc, "xh", [xhT])
    _tap(nc, "q", qT)
    _tap(nc, "k", kT)
    _tap(nc, "v", v3p)


    xrb_sb = {}

    def prefetch_resid(t):
        # x + bo', computed off the LN2 critical chain (bo is exact here:
        # it joins x2 before LN2, as in the reference).
        xr = xrp.tile([P, D], f32, tag="xr", name="xr")
        nc.sync.dma_start(out=xr, in_=xd[ts(t, P), :])
        xrb_sb[t] = xr
        nc.gpsimd.tensor_add(out=xr, in0=xr, in1=bo_b)

    # ---------- attention for one (head, chunk) ----------
    def attn(h, c):
        g, hp = h // 3, h % 3
        base = 32 * hp
        ees = []
        for jj in range(4):
            sps = ps_s()
            for jp in range(2):
                j = 2 * jj + jp
                nc.tensor.matmul(
                    sps[:, jp, :],
                    kT[g][base:base + 32, :, ts(j, P)],
                    qT[g][base:base + 32, :, ts(c, CW)],
                    start=True, stop=True, perf_mode=DR)
            ee = eep.tile([P, 2, CW], e5, tag="e", name="ee")
            nc.scalar.activation(out=ee, in_=sps, func=AF.Exp)
            ees.append(ee)
        cps = ps_c()[:, 0:4 * (DH + 1)].rearrange("p (q d) -> p q d",
                                                  d=DH + 1)
        for jj in range(4):
            for qt in range(4):
                nc.tensor.matmul(
                    cps[:, qt, :],
                    ees[jj][:, :, ts(qt, P)],
                    v3p[jj][:, :, h, :],
                    start=(jj == 0 and qt == 0),
                    stop=(jj == 3 and qt == 3),
                    perf_mode=DR, skip_group_check=True)
        rc = rcp.tile([P, 4], f32, tag="rc", name="rc")
        with tc.high_priority():
            nc.vector.reciprocal(out=rc, in_=cps[:, :, DH:DH + 1])
            nc.vector.tensor_tensor(
                out=ctx_sb[:, 4 * c:4 * c + 4, h * DH:(h + 1) * DH],
                in0=cps[:, :, 0:DH], in1=_bcast_free(rc[:, :], DH),
                op=ALU.mult)

    # ---------- Wo + residual + LN2 + transposes for one token tile ----------
    def wo_ln2(t, tail=False):
        trp = ps_mm
        trc = _tr_view(trp())
        for j in range(DT):
            nc.tensor.matmul(trc[:, j:j + 1, :], ctx_sb[:, t, ts(j, P)],
                             ident, is_transpose=True, start=(j == 0),
                             stop=(j == DT - 1), skip_group_check=True)
        nc.vector.tensor_copy(out=ctxT[:, :, ts(t, P)], in_=trc)
        xrb = xrb_sb[t]
        for lo, w in ((0, CW), (CW, 256)):
            mm = ps_mm()
            for j2 in range(3):
                nc.tensor.matmul(mm[:, 0:w],
                                 ctxT[:, 2 * j2:2 * j2 + 2, ts(t, P)],
                                 wo_sb[:, 2 * j2:2 * j2 + 2, lo:lo + w],
                                 start=(j2 == 0), stop=(j2 == 2),
                                 perf_mode=DR)
            nc.vector.scalar_tensor_tensor(
                out=x2_sb[t][:, lo:lo + w], in0=mm[:, 0:w], scalar=1.0,
                in1=xrb[:, lo:lo + w], op0=ALU.mult, op1=ALU.add)
        # LN2: stats + Newton rsqrt on DVE (ACT stays on the Exp table).
        stats = ln_pool.tile([P, 3, 6], f32, tag="st2", name="st2")
        for s3 in range(3):
            nc.vector.bn_stats(out=stats[:, s3, :],
                               in_=x2_sb[t][:, s3 * 256:(s3 + 1) * 256])
        mv = ln_pool.tile([P, 2], f32, tag="mv2", name="mv2")
        nc.vector.bn_aggr(out=mv, in_=stats)
        y = _newton_rstd(mv, "2")
        xc32 = xc_pool.tile([P, D], f32, tag="xc32", name="xc32")
        nc.vector.tensor_scalar(out=xc32, in0=x2_sb[t], scalar1=mv[:, 0:1],
                                scalar2=y, op0=ALU.subtract, op1=ALU.mult)
        xch = xc_pool.tile([P, D], e4, tag="xch", name="xch")
        nc.gpsimd.tensor_copy(out=xch, in_=xc32)
        xcl = xc_pool.tile([P, D], e4, tag="xcl", name="xcl")
        nc.gpsimd.tensor_tensor(out=xcl, in0=xc32, in1=xch, op=ALU.subtract)
        # Separate hi/lo transpose groups: the hi path (which gates fc1's
        # first terms) doesn't wait for xcl.
        trh = _tr_view(trp())
        for j in range(DT):
            nc.tensor.matmul(trh[:, j:j + 1, :], xch[:, ts(j, P)], ident,
                             is_transpose=True, start=(j == 0),
                             stop=(j == DT - 1), skip_group_check=True)
        nc.vector.tensor_copy(out=x2hT_h[:, :, ts(t, P)], in_=trh)
        trl = _tr_view(trp())
        for j in range(DT):
            nc.tensor.matmul(trl[:, j:j + 1, :], xcl[:, ts(j, P)], ident,
                             is_transpose=True, start=(j == 0),
                             stop=(j == DT - 1), skip_group_check=True)
        nc.vector.tensor_copy(out=x2hT_l[:, :, ts(t, P)], in_=trl)

    # ---------- fc1 for one (m, chunk); gT tiles are per-chunk ----------
    def fc1(m, c, gTh):
        w1h = w1p.tile([P, DT, P], e4, tag="w1h", name="w1h")
        nc.sync.dma_start(out=w1h, in_=w1hd[m])
        w1l = w1p.tile([P, DT, P], e5, tag="w1l", name="w1l")
        nc.sync.dma_start(out=w1l, in_=w1ld[m])
        mm = ps_mm()
        terms = ((w1h, x2hT_h), (w1l, x2hT_h), (w1h, x2hT_l))
        for ti, (wt, at) in enumerate(terms):
            for m2 in range(3):
                nc.tensor.matmul(
                    mm, wt[:, 2 * m2:2 * m2 + 2, :],
                    at[:, 2 * m2:2 * m2 + 2, ts(c, CW)],
                    start=(ti == 0 and m2 == 0), stop=(ti == 2 and m2 == 2),
                    perf_mode=DR)
        nc.scalar.activation(out=gTh[:, m, :], in_=mm, func=AF.Gelu,
                             bias=b1_sb[:, m:m + 1])

    # ---------- fc2 + residual + out for one token tile ----------
    def fc2(t, gTh):
        tc_ = t % 4  # column within the chunk's gT tiles
        nc.gpsimd.tensor_add(out=x2_sb[t], in0=x2_sb[t], in1=b2_b)
        ot = outp.tile([P, D], f32, tag="ot", name="ot")
        for lo, w in ((0, CW), (CW, 256)):
            mm = ps_mm()
            terms = ((gTh, w2_h), (gTh, w2_l))
            for ti, (gt, wt) in enumerate(terms):
                for u2 in range(UT // 2):
                    nc.tensor.matmul(
                        mm[:, 0:w],
                        gt[:, 2 * u2:2 * u2 + 2, ts(tc_, P)],
                        wt[:, 2 * u2:2 * u2 + 2, lo:lo + w],
                        start=(ti == 0 and u2 == 0),
                        stop=(ti == 1 and u2 == UT // 2 - 1),
                        perf_mode=DR)
            nc.vector.scalar_tensor_tensor(
                out=ot[:, lo:lo + w], in0=mm[:, 0:w], scalar=1.0,
                in1=x2_sb[t][:, lo:lo + w], op0=ALU.mult, op1=ALU.add)
        nc.sync.dma_start(out=outd[ts(t, P), :], in_=ot)

    # ---------- schedule ----------
    def gT_pair():
        return (gp.tile([P, UT, CW], e4, tag="gh", bufs=1, name="gh"),)

    for t in range(4):
        prefetch_resid(t)
    for h in range(H):
        attn(h, 0)
    gT0 = gT_pair()
    for h in range(H):
        attn(h, 1)
        if h < 4:
            wo_ln2(h)            # chunk-0 tiles 0..3 (no ACT work inside)
    for m in range(UT):
        fc1(m, 0, *gT0)          # single Exp->Gelu table transition
    _tap(nc, "ctx", [ctx_sb])
    for t in range(4, NT):
        prefetch_resid(t)
    for t in range(4, NT):
        wo_ln2(t, tail=True)     # chunk-1 tiles (transposes on idle s banks)
    for t0 in range(4):
        fc2(t0, *gT0)            # chunk-0 fc2 (gT0 frees for reuse)
    _tap(nc, "x2", x2_sb)
    gT1 = gT_pair()
    for m in range(UT):
        fc1(m, 1, *gT1)
    _tap(nc, "g", [gT1[0]])
    for t in range(4, NT):
        fc2(t, *gT1)

    pl.close("outp", "w1p", "gp", "bigB", "xr", "rc", "ee", "bigA")


def _get_program():
    global _PROGRAM
    if _PROGRAM is None:
        _PROGRAM = _build_program()
    return _PROGRAM


def _prepare_host_inputs(inputs):
    f64 = np.float64
    e4n = ml_dtypes.float8_e4m3
    e5n = ml_dtypes.float8_e5m2
    x = np.asarray(inputs["x"], np.float32)
    g1 = np.asarray(inputs["ln1_g"], f64)
    b1l = np.asarray(inputs["ln1_b"], f64)
    g2 = np.asarray(inputs["ln2_g"], f64)
    b2l = np.asarray(inputs["ln2_b"], f64)
    Wq = np.asarray(inputs["Wq"], f64)
    Wk = np.asarray(inputs["Wk"], f64)
    Wv = np.asarray(inputs["Wv"], f64)
    Wo = np.asarray(inputs["Wo"], f64)
    W1 = np.asarray(inputs["W1"], f64)
    W2 = np.asarray(inputs["W2"], f64)
    bq = np.asarray(inputs["bq"], f64)
    bv = np.asarray(inputs["bv"], f64)
    bo = np.asarray(inputs["bo"], f64)
    b1 = np.asarray(inputs["b1"], f64)
    b2 = np.asarray(inputs["b2"], f64)

    def q8(a, dt):
        return np.ascontiguousarray(np.asarray(a, np.float32)).astype(dt)

    def split(a):
        hi = q8(a, e4n)
        lo = q8(np.asarray(a, np.float32) - hi.astype(np.float32), e5n)
        return hi, lo

    Wqf = (g1[:, None] * Wq * 0.125).astype(np.float32)
    Wkf = (g1[:, None] * Wk).astype(np.float32)
    bqf = ((b1l @ Wq + bq) * 0.125).astype(np.float32)
    Wvf = (g1[:, None] * Wv).astype(np.float32)
    bvf = b1l @ Wv + bv
    bof = (bo + bvf @ Wo).astype(np.float32)
    W1f = (g2[:, None] * W1).astype(np.float32)
    b1f = (b2l @ W1 + b1).astype(np.float32)

    # q/k out-tile column permutation: slot (g, s) partition p = 32h'+i maps
    # to feature (3g+h')*64 + 32s + i (scores contract the two 32-wide d
    # halves as DoubleRow subtiles at base partitions 0/32/64; 3 heads per
    # 128-partition group, partitions 96:128 unused).
    perm = np.empty((4, 2, 96), np.int64)
    for g in range(4):
        for s in range(2):
            for hp in range(3):
                for i in range(32):
                    perm[g, s, 32 * hp + i] = (3 * g + hp) * 64 + 32 * s + i
    wqk = np.empty((P, DT, 16, 96), np.float32)
    bqh = np.zeros((P, 8), np.float32)
    for g in range(4):
        for s in range(2):
            idx = g * 2 + s
            cols = perm[g, s]
            wqk[:, :, idx, :] = Wqf[:, cols].reshape(DT, P, 96).transpose(
                1, 0, 2)
            wqk[:, :, 8 + idx, :] = Wkf[:, cols].reshape(DT, P, 96).transpose(
                1, 0, 2)
            bqh[0:96, idx] = bqf[cols]
    wqk_h, wqk_l = split(wqk)

    wv_h = q8(Wvf.reshape(DT, P, D).transpose(1, 0, 2), e4n)
    wo_h = q8(np.asarray(Wo, np.float32).reshape(DT, P, D).transpose(
        1, 0, 2), e4n)
    w1_pack = W1f.reshape(DT, P, UT, P).transpose(2, 1, 0, 3)  # [m, p, j, o]
    w1_h, w1_l = split(np.ascontiguousarray(w1_pack))
    w2_pack = np.asarray(W2, np.float32).reshape(UT, P, D).transpose(1, 0, 2)
    w2_h, w2_l = split(np.ascontiguousarray(w2_pack))  # [p, u, f]

    b1h = np.ascontiguousarray(b1f.reshape(UT, P).T)
    bob_h = np.ascontiguousarray(
        np.broadcast_to(bof.astype(np.float32), (P, D)))
    b2b_h = np.ascontiguousarray(
        np.broadcast_to(b2.astype(np.float32), (P, D)))

    shared = {"wqkh": wqk_h, "wqkl": wqk_l, "wv": wv_h, "wo": wo_h,
              "w1h": w1_h, "w1l": w1_l, "w2h": w2_h, "w2l": w2_l,
              "bq": bqh, "b1": b1h, "bob": bob_h, "b2b": b2b_h}
    return x, shared


def _run_once(inputs):
    x, shared = _prepare_host_inputs(inputs)
    nc = _get_program()
    in_maps = [dict(shared, x=np.ascontiguousarray(x[c]))
               for c in range(NCORES)]
    import time
    t0 = time.perf_counter()
    res = run_bass_kernel_spmd(nc, in_maps, list(range(NCORES)))
    t1 = time.perf_counter()
    kernel._last_wall_s = t1 - t0
    out = np.stack([res.results[c]["out"] for c in range(NCORES)], axis=0)
    return out.astype(np.float32)


def _run_subprocess(inputs):
    """Re-run in a fresh process: a wedged NeuronCore (intermittent
    NRT_EXEC_UNIT_UNRECOVERABLE) recovers with a fresh axon session, while
    in-process retries keep failing."""
    import subprocess, sys, tempfile, os
    d = tempfile.mkdtemp()
    inp, outp = os.path.join(d, "in.npz"), os.path.join(d, "out.npy")
    np.savez(inp, **inputs)
    code = (
        "import numpy as np, importlib.util, sys\n"
        f"spec = importlib.util.spec_from_file_location('kmod', {__file__!r})\n"
        "m = importlib.util.module_from_spec(spec); spec.loader.exec_module(m)\n"
        f"ins = dict(np.load({inp!r}))\n"
        f"np.save({outp!r}, m._run_once(ins))\n"
    )
    subprocess.run([sys.executable, "-c", code], check=True, timeout=900)
    return np.load(outp)


def kernel(**inputs):
    import time
    last_err = None
    for attempt in range(2):
        try:
            return _run_once(inputs)
        except Exception as e:  # transient NRT device wedge: retry
            last_err = e
            time.sleep(2.0 * (attempt + 1))
    # In-process retries exhausted: the core is likely wedged; fresh
    # processes (new axon session) recover it.
    for attempt in range(4):
        try:
            out = _run_subprocess(inputs)
            kernel._last_wall_s = float("nan")
            return out
        except Exception as e:
            last_err = e
            time.sleep(3.0 * (attempt + 1))
    raise last_err
